# revision 20
# baseline (speedup 1.0000x reference)
"""GPT-2-small forward on 8 trn2 NeuronCores, data-parallel over batch.

Layout convention on device: activations are stored transposed,
[feature -> partitions (tiles of 128), token -> free dim].  Every matmul is
out = lhsT.T @ rhs with the contraction dim on partitions, so no transposes
are ever needed on device:
  - k/q:    lhsT = W.T tile (stationary), rhs = h_T        -> out [feat, tok]
  - v:      lhsT = h_T tile (stationary), rhs = W.T slice  -> out [tok, feat]
  - scores: lhsT = k_T tile, rhs = q_T                     -> S_T [tk, tq]
  - y:      lhsT = v tile (with a fused ones column so the softmax
            denominator falls out as psum row 64), rhs = exp(S_T)
Softmax reductions over tk (partitions) ride the tensor engine; LN stats do
too (ones-vector matmuls).  Residual stream stays fp32 in SBUF the whole
forward; matmul operands are bf16.  The last layer only computes
q/attention/proj/MLP for the final token (that is all the lm head reads).
"""

import sys

sys.path.insert(0, "/opt/trn_rl_repo")

from contextlib import ExitStack

import ml_dtypes
import numpy as np

import concourse.bass as bass
import concourse.tile as tile
from concourse import bacc, mybir
from concourse.bass_utils import run_bass_kernel_spmd

BF16NP = ml_dtypes.bfloat16
F32 = mybir.dt.float32
BF = mybir.dt.bfloat16

B, T, E, H, L, V = 8, 1024, 768, 12, 12, 50304
KT = E // 128          # 6 k-tiles of the embedding dim
D = E // H             # 64 head dim
F = 4 * E              # 3072
FT = F // 128          # 24
NVC = (V + 511) // 512  # 99 wte chunks of 512 vocab cols
VP = NVC * 512          # 50688 padded vocab
TS_FULL = [(0, 512), (512, 512)]
TS_LAST = [(1023, 1)]


# --------------------------------------------------------------------------
# host-side weight prep (cached across kernel() calls)
# --------------------------------------------------------------------------

def _lhsT(w):
    """[O, I] fp32 -> [128, I//128, O] bf16 (lhsT-ready, partition major)."""
    I = w.shape[1]
    return np.ascontiguousarray(
        w.T.reshape(I // 128, 128, -1).transpose(1, 0, 2).astype(BF16NP)
    )


def _pcol(b, nt):
    """per-feature vector [nt*128] -> [128, nt] (partition major)."""
    return np.ascontiguousarray(np.asarray(b, np.float32).reshape(nt, 128).T)


_prep_cache = {}


def _prep_weights(inp):
    key = (id(inp["wte"]), id(inp["qkv_w"]))
    if key in _prep_cache:
        return _prep_cache[key][1]

    qkvw = np.stack([_lhsT(inp["qkv_w"][l]) for l in range(L)])        # [L,128,6,2304]
    qkvw = np.ascontiguousarray(
        qkvw.reshape(L, 128, KT, 3, 768).transpose(0, 3, 1, 2, 4)
    )                                                                   # [L,3,128,6,768]
    projw = np.stack([_lhsT(inp["proj_w"][l]) for l in range(L)])       # [L,128,6,768]
    c1w = np.stack([_lhsT(inp["c1_w"][l]) for l in range(L)])           # [L,128,6,3072]
    c1w = np.ascontiguousarray(
        c1w.reshape(L, 128, KT, 4, 768).transpose(0, 3, 1, 2, 4)
    )                                                                   # [L,4,128,6,768]
    c2w = np.stack([_lhsT(inp["c2_w"][l]) for l in range(L)])           # [L,128,24,768]
    c2w = np.ascontiguousarray(
        c2w.reshape(L, 128, FT, 6, 128).transpose(0, 3, 1, 2, 4)
    )                                                                   # [L,6,128,24,128]

    wteT = np.asarray(inp["wte"], np.float32).T.astype(BF16NP)          # [768, V]
    wteT = np.concatenate([wteT, np.zeros((E, VP - V), BF16NP)], axis=1)
    wteT = np.ascontiguousarray(
        wteT.reshape(KT, 128, NVC, 512).transpose(2, 1, 0, 3)
    )                                                                   # [99,128,6,512]

    qkv_b = np.asarray(inp["qkv_b"], np.float32)
    dev = dict(
        qkvw=qkvw, projw=projw, c1w=c1w, c2w=c2w, wteT=wteT,
        kqb=np.stack([_pcol(qkv_b[l, :1536], 12) for l in range(L)]),
        vb=np.ascontiguousarray(qkv_b[:, 1536:]),                       # [L,768]
        projb=np.stack([_pcol(inp["proj_b"][l], 6) for l in range(L)]),
        c1b=np.stack([_pcol(inp["c1_b"][l], 24) for l in range(L)]),
        c2b=np.stack([_pcol(inp["c2_b"][l], 6) for l in range(L)]),
        lnw=np.stack([_pcol(inp["ln_w"][l], 6) for l in range(L)]),
        lnb=np.stack([_pcol(inp["ln_b"][l], 6) for l in range(L)]),
        lnfw=_pcol(inp["lnf_w"], 6),
        lnfb=_pcol(inp["lnf_b"], 6),
    )
    _prep_cache.clear()
    _prep_cache[key] = ((inp["wte"], inp["qkv_w"]), dev)  # hold refs so ids stay valid
    return dev


# --------------------------------------------------------------------------
# device program
# --------------------------------------------------------------------------

def _emit_ln(nc, pools, x_T, w_sb, kt_w, b_sb, kt_b, h_bf, slices, eps_sb, ones_bf,
             onesf=None):
    """h_bf[:, :, sl] = LN(x_T[:, :, sl]) * w + b   (bf16 out)."""
    A = mybir.AluOpType
    for (t0, tn) in slices:
        ssum = pools["sp"].tile([1, 512], F32, tag="sp", name="ssum")
        qsum = pools["sp"].tile([1, 512], F32, tag="sp", name="qsum")
        for kt in range(KT):
            xbt = pools["xb"].tile([128, 512], BF, tag="xb", name="xbt")
            nc.vector.tensor_copy(out=xbt[:, :tn], in_=x_T[:, kt, t0:t0 + tn])
            sqt = pools["xb"].tile([128, 512], BF, tag="sq", name="sqt")
            nc.vector.tensor_mul(sqt[:, :tn], xbt[:, :tn], xbt[:, :tn])
            nc.tensor.matmul(ssum[0:1, :tn], ones_bf[:, 0:1], xbt[:, :tn],
                             start=(kt == 0), stop=(kt == KT - 1))
            nc.tensor.matmul(qsum[0:1, :tn], ones_bf[:, 0:1], sqt[:, :tn],
                             start=(kt == 0), stop=(kt == KT - 1))
        mean = pools["stat"].tile([1, 512], F32, tag="stat", name="mean")
        nc.scalar.activation(mean[:, :tn], ssum[0:1, :tn],
                             mybir.ActivationFunctionType.Copy, scale=1.0 / E)
        ex2 = pools["stat"].tile([1, 512], F32, tag="stat", name="ex2")
        nc.scalar.activation(ex2[:, :tn], qsum[0:1, :tn],
                             mybir.ActivationFunctionType.Copy, scale=1.0 / E)
        var = pools["stat"].tile([1, 512], F32, tag="stat", name="var")
        nc.vector.scalar_tensor_tensor(
            out=var[:, :tn], in0=mean[:, :tn], scalar=-1.0, in1=mean[:, :tn],
            op0=A.mult, op1=A.mult)                                      # -mean^2
        nc.vector.tensor_add(var[:, :tn], var[:, :tn], ex2[:, :tn])
        sdv = pools["stat"].tile([1, 512], F32, tag="stat", name="sdv")
        nc.scalar.activation(sdv[:, :tn], var[:, :tn],
                             mybir.ActivationFunctionType.Sqrt,
                             bias=eps_sb[0:1, 0:1])
        rstd = pools["stat"].tile([1, 512], F32, tag="stat", name="rstd")
        nc.vector.reciprocal(rstd[:, :tn], sdv[:, :tn])
        # broadcast across partitions via K=1 outer product with a ones column
        mbc = pools["bcp"].tile([128, 512], F32, tag="bc", name="mbc")
        nc.tensor.matmul(mbc[:, :tn], onesf[0:1, :], mean[0:1, :tn],
                         start=True, stop=True)
        rbc = pools["bcp"].tile([128, 512], F32, tag="bc", name="rbc")
        nc.tensor.matmul(rbc[:, :tn], onesf[0:1, :], rstd[0:1, :tn],
                         start=True, stop=True)
        for kt in range(KT):
            tt = pools["tmp"].tile([128, 512], F32, tag="tmp", name="tt")
            nc.vector.tensor_sub(tt[:, :tn], x_T[:, kt, t0:t0 + tn], mbc[:, :tn])
            nc.vector.scalar_tensor_tensor(
                out=tt[:, :tn], in0=tt[:, :tn],
                scalar=w_sb[:, kt_w + kt:kt_w + kt + 1],
                in1=rbc[:, :tn], op0=A.mult, op1=A.mult)
            nc.vector.tensor_scalar_add(
                h_bf[:, kt, t0:t0 + tn], tt[:, :tn],
                b_sb[:, kt_b + kt:kt_b + kt + 1])


def _emit_layer(nc, pools, dram, x_T, eps_sb, ones_bf, masks, l, tq_slices,
                onesf=None):
    pp, wp, sm, act = pools["pp"], pools["w"], pools["sm"], pools["act"]
    A = mybir.AluOpType

    # per-layer small params
    lnp = sm.tile([128, 12], F32, tag="lnp", name="lnp")   # cols 0:6 w, 6:12 b
    nc.sync.dma_start(out=lnp[:, 0:6], in_=dram["lnw"][l])
    nc.sync.dma_start(out=lnp[:, 6:12], in_=dram["lnb"][l])
    kqb = sm.tile([128, 12], F32, tag="kqb", name="kqb")
    nc.sync.dma_start(out=kqb, in_=dram["kqb"][l])
    pcb = sm.tile([128, 12], F32, tag="pcb", name="pcb")   # 0:6 projb, 6:12 c2b
    nc.sync.dma_start(out=pcb[:, 0:6], in_=dram["projb"][l])
    nc.sync.dma_start(out=pcb[:, 6:12], in_=dram["c2b"][l])
    c1b = sm.tile([128, 24], F32, tag="c1b", name="c1b")
    nc.sync.dma_start(out=c1b, in_=dram["c1b"][l])
    vbc = sm.tile([128, 768], F32, tag="vbc", bufs=1, name="vbc")
    nc.sync.dma_start(out=vbc, in_=dram["vb"][l:l + 1, :].to_broadcast([128, 768]))

    # ---------------- LN1 -> h (full T: k and v always need every token)
    h_bf = act.tile([128, KT, T], BF, tag="h", bufs=1, name="h_bf")
    _emit_ln(nc, pools, x_T, lnp, 0, lnp, 6, h_bf, TS_FULL, eps_sb, ones_bf, onesf)

    # ---------------- qkv
    k_T = act.tile([128, KT, T], BF, tag="k", bufs=1, name="k_T")
    q_T = act.tile([128, KT, T], BF, tag="q", bufs=1, name="q_T")
    for ci, dst, bo, slices in ((0, k_T, 0, TS_FULL), (1, q_T, 6, tq_slices)):
        w_t = wp.tile([128, KT, 768], BF, tag="w", name="wkq")
        nc.sync.dma_start(out=w_t, in_=dram["qkvw"][l, ci])
        for o in range(6):
            for (t0, tn) in slices:
                ps = pp.tile([128, 512], F32, tag="pp", name="ps_kq")
                for kt in range(KT):
                    nc.tensor.matmul(ps[:, :tn], w_t[:, kt, o * 128:(o + 1) * 128],
                                     h_bf[:, kt, t0:t0 + tn],
                                     start=(kt == 0), stop=(kt == KT - 1))
                nc.vector.tensor_scalar_add(
                    dst[:, o, t0:t0 + tn], ps[:, :tn], kqb[:, bo + o:bo + o + 1])

    # v in [token, head, d] layout
    wv = wp.tile([128, KT, 768], BF, tag="w", name="wv")
    nc.sync.dma_start(out=wv, in_=dram["qkvw"][l, 2])
    v_sb = act.tile([128, 8, H, 64], BF, tag="v", bufs=1, name="v_sb")
    for tt in range(8):
        for (o0, on, h0, hn) in ((0, 512, 0, 8), (512, 256, 8, 4)):
            ps = pp.tile([128, 512], F32, tag="pp", name="ps_v")
            for kt in range(KT):
                nc.tensor.matmul(ps[:, :on], h_bf[:, kt, tt * 128:(tt + 1) * 128],
                                 wv[:, kt, o0:o0 + on],
                                 start=(kt == 0), stop=(kt == KT - 1))
            nc.vector.tensor_add(
                v_sb[:, tt, h0:h0 + hn, :],
                ps[:, :on].rearrange("p (h d) -> p h d", d=64),
                vbc[:, o0:o0 + on].rearrange("p (h d) -> p h d", d=64))

    # ---------------- attention + proj (per tq slice)
    wpj = wp.tile([128, KT, 768], BF, tag="w", name="wpj")
    nc.sync.dma_start(out=wpj, in_=dram["projw"][l])
    for (t0, tn) in tq_slices:
        rmax = (t0 + tn - 1) // 128
        y_T = act.tile([128, KT, 512], BF, tag="y", bufs=1, name="y_T")
        for hp in range(6):
            ea = act.tile([128, 8, 512], BF, tag="ea", bufs=1, name="ea")
            eb = act.tile([128, 8, 512], BF, tag="eb", bufs=1, name="eb")
            for half, ex in ((0, ea), (1, eb)):
                p0 = 64 * half
                for r in range(rmax + 1):
                    ps = pp.tile([128, 512], F32, tag="pp", name="ps_s")
                    nc.tensor.matmul(
                        ps[:, :tn],
                        k_T[p0:p0 + 64, hp, r * 128:(r + 1) * 128],
                        q_T[p0:p0 + 64, hp, t0:t0 + tn],
                        start=True, stop=True)
                    nc.scalar.activation(ex[:, r, :tn], ps[:, :tn],
                                         mybir.ActivationFunctionType.Exp)
                    m = r - (t0 // 128)  # mask variant; >=0 only on diagonal blocks
                    if tn > 1 and m >= 0:
                        nc.vector.tensor_mul(ex[:, r, :tn], ex[:, r, :tn],
                                             masks[:, m, :tn])
            # softmax denominators (ones-matmul over tk partitions)
            dens = []
            for half, ex in ((0, ea), (1, eb)):
                den = pools["sp"].tile([1, 512], F32, tag="sp", name="den")
                for r in range(rmax + 1):
                    nc.tensor.matmul(den[0:1, :tn], ones_bf[:, 0:1], ex[:, r, :tn],
                                     start=(r == 0), stop=(r == rmax))
                dens.append(den)
            # y for both heads, col-packed: head a -> psum rows 0:64,
            # head b -> rows 64:128
            py = pp.tile([128, 512], F32, tag="pp", name="ps_y")
            for r in range(rmax + 1):
                nc.tensor.matmul(py[0:64, :tn], v_sb[:, r, 2 * hp, :],
                                 ea[:, r, :tn], start=(r == 0), stop=(r == rmax))
                nc.tensor.matmul(py[64:128, :tn], v_sb[:, r, 2 * hp + 1, :],
                                 eb[:, r, :tn], start=(r == 0), stop=(r == rmax),
                                 tile_position=(0, 64))
            bca = pools["bcp"].tile([128, 512], F32, tag="bc", name="bca")
            for half in (0, 1):
                rcp = pools["rcp"].tile([1, 512], F32, tag="rcp", name="rcp")
                nc.vector.reciprocal(rcp[:, :tn], dens[half][0:1, :tn])
                nc.tensor.matmul(bca[64 * half:64 * half + 64, :tn],
                                 onesf[0:1, 0:64], rcp[0:1, :tn],
                                 start=True, stop=True,
                                 tile_position=(0, 64 * half))
            bcs = pools["xb"].tile([128, 512], F32, tag="bcs", name="bcs")
            nc.scalar.activation(bcs[:, :tn], bca[:, :tn],
                                 mybir.ActivationFunctionType.Copy)
            nc.vector.tensor_mul(y_T[0:64, hp, :tn], py[0:64, :tn],
                                 bcs[0:64, :tn])
            nc.vector.tensor_mul(y_T[64:128, hp, :tn], py[64:128, :tn],
                                 bcs[64:128, :tn])

        for o in range(6):  # proj + residual
            ps = pp.tile([128, 512], F32, tag="pp", name="ps_p")
            for kt in range(KT):
                nc.tensor.matmul(ps[:, :tn], wpj[:, kt, o * 128:(o + 1) * 128],
                                 y_T[:, kt, :tn],
                                 start=(kt == 0), stop=(kt == KT - 1))
            nc.vector.scalar_tensor_tensor(
                out=x_T[:, o, t0:t0 + tn], in0=ps[:, :tn],
                scalar=pcb[:, o:o + 1], in1=x_T[:, o, t0:t0 + tn],
                op0=A.add, op1=A.add)

    # ---------------- LN2 + MLP (slice-major; c1/c2 weights re-streamed per
    # slice to keep only one mid buffer live -- DMA is far from the bottleneck)
    h2 = act.tile([128, KT, T], BF, tag="h", bufs=1, name="h2")
    _emit_ln(nc, pools, x_T, lnp, 0, lnp, 6, h2, tq_slices, eps_sb, ones_bf, onesf)
    for (t0, tn) in tq_slices:
        mid = pools["mid"].tile([128, FT, 512], BF, tag="mid", name="mid")
        for ci in range(4):
            wc = wp.tile([128, KT, 768], BF, tag="w", name="wc1")
            nc.sync.dma_start(out=wc, in_=dram["c1w"][l, ci])
            for oo in range(6):
                o = ci * 6 + oo
                ps = pp.tile([128, 512], F32, tag="pp", name="ps_c1")
                for kt in range(KT):
                    nc.tensor.matmul(ps[:, :tn], wc[:, kt, oo * 128:(oo + 1) * 128],
                                     h2[:, kt, t0:t0 + tn],
                                     start=(kt == 0), stop=(kt == KT - 1))
                nc.scalar.activation(mid[:, o, :tn], ps[:, :tn],
                                     mybir.ActivationFunctionType.Gelu,
                                     bias=c1b[:, o:o + 1])
        for o in range(6):
            wc2 = wp.tile([128, FT, 128], BF, tag="w", name="wc2")
            nc.sync.dma_start(out=wc2, in_=dram["c2w"][l, o])
            ps = pp.tile([128, 512], F32, tag="pp", name="ps_c2")
            for kt in range(FT):
                nc.tensor.matmul(ps[:, :tn], wc2[:, kt, :], mid[:, kt, :tn],
                                 start=(kt == 0), stop=(kt == FT - 1))
            nc.vector.scalar_tensor_tensor(
                out=x_T[:, o, t0:t0 + tn], in0=ps[:, :tn],
                scalar=pcb[:, 6 + o:7 + o], in1=x_T[:, o, t0:t0 + tn],
                op0=A.add, op1=A.add)


def _build_program():
    nc = bacc.Bacc("TRN2", target_bir_lowering=False, debug=False,
                   enable_asserts=False, num_devices=8)
    d = {}
    d["x0"] = nc.dram_tensor("x0", [128, KT, T], F32, kind="ExternalInput").ap()
    d["qkvw"] = nc.dram_tensor("qkvw", [L, 3, 128, KT, 768], BF, kind="ExternalInput").ap()
    d["projw"] = nc.dram_tensor("projw", [L, 128, KT, 768], BF, kind="ExternalInput").ap()
    d["c1w"] = nc.dram_tensor("c1w", [L, 4, 128, KT, 768], BF, kind="ExternalInput").ap()
    d["c2w"] = nc.dram_tensor("c2w", [L, 6, 128, FT, 128], BF, kind="ExternalInput").ap()
    d["wteT"] = nc.dram_tensor("wteT", [NVC, 128, KT, 512], BF, kind="ExternalInput").ap()
    for nm, shp in (("kqb", [L, 128, 12]), ("vb", [L, 768]),
                    ("projb", [L, 128, 6]), ("c1b", [L, 128, 24]),
                    ("c2b", [L, 128, 6]), ("lnw", [L, 128, 6]),
                    ("lnb", [L, 128, 6]), ("lnfw", [128, 6]), ("lnfb", [128, 6])):
        d[nm] = nc.dram_tensor(nm, shp, F32, kind="ExternalInput").ap()
    logits_d = nc.dram_tensor("logits", [1, VP], F32, kind="ExternalOutput").ap()

    with tile.TileContext(nc) as tc, ExitStack() as ctx:
        pools = {
            "pp": ctx.enter_context(tc.tile_pool(name="pp", bufs=4, space="PSUM")),
            "sp": ctx.enter_context(tc.tile_pool(name="sp", bufs=2, space="PSUM")),
            "bcp": ctx.enter_context(tc.tile_pool(name="bcp", bufs=2, space="PSUM")),
            "w": ctx.enter_context(tc.tile_pool(name="w", bufs=3)),
            "act": ctx.enter_context(tc.tile_pool(name="act", bufs=2)),
            "mid": ctx.enter_context(tc.tile_pool(name="mid", bufs=1)),
            "xb": ctx.enter_context(tc.tile_pool(name="xb", bufs=2)),
            "stat": ctx.enter_context(tc.tile_pool(name="stat", bufs=3)),
            "rcp": ctx.enter_context(tc.tile_pool(name="rcp", bufs=2)),
            "tmp": ctx.enter_context(tc.tile_pool(name="tmp", bufs=2)),
            "sm": ctx.enter_context(tc.tile_pool(name="sm", bufs=2)),
            "const": ctx.enter_context(tc.tile_pool(name="const", bufs=1)),
        }
        cp = pools["const"]
        x_T = cp.tile([128, KT, T], F32, tag="x", name="x_T")
        nc.sync.dma_start(out=x_T, in_=d["x0"])
        ones_bf = cp.tile([128, 1], BF, tag="ones", name="ones_bf")
        nc.vector.memset(ones_bf, 1.0)
        onesf = cp.tile([65, 128], F32, tag="onesf", name="onesf")
        nc.vector.memset(onesf, 1.0)
        eps_sb = cp.tile([1, 1], F32, tag="eps", name="eps_sb")
        nc.vector.memset(eps_sb, 1e-5)
        masks = cp.tile([128, 4, 512], BF, tag="masks", name="masks")
        nc.vector.memset(masks, 1.0)
        for m in range(4):
            # keep 1.0 iff tq - tk >= 0, i.e. -x + y - 128*m >= 0
            nc.gpsimd.affine_select(
                out=masks[:, m, :], in_=masks[:, m, :],
                compare_op=mybir.AluOpType.is_ge, fill=0.0,
                base=-128 * m, pattern=[[1, 512]], channel_multiplier=-1)

        for l in range(L):
            _emit_layer(nc, pools, d, x_T, eps_sb, ones_bf, masks, l,
                        TS_FULL if l < L - 1 else TS_LAST, onesf)

        # final LN on last token only
        lnf = pools["sm"].tile([128, 12], F32, tag="lnf", name="lnf")
        nc.sync.dma_start(out=lnf[:, 0:6], in_=d["lnfw"])
        nc.sync.dma_start(out=lnf[:, 6:12], in_=d["lnfb"])
        hf = pools["act"].tile([128, KT, T], BF, tag="h", bufs=1, name="hf")
        _emit_ln(nc, pools, x_T, lnf, 0, lnf, 6, hf, TS_LAST, eps_sb, ones_bf, onesf)

        # lm head: [1,768] @ [768, 50688]
        for c in range(NVC):
            wt = pools["w"].tile([128, KT, 512], BF, tag="w", name="wt")
            nc.sync.dma_start(out=wt, in_=d["wteT"][c])
            ps = pools["sp"].tile([1, 512], F32, tag="sp", name="ps_lm")
            for kt in range(KT):
                nc.tensor.matmul(ps[0:1, :], hf[:, kt, 1023:1024], wt[:, kt, :],
                                 start=(kt == 0), stop=(kt == KT - 1))
            ob = pools["stat"].tile([1, 512], F32, tag="ob", bufs=2, name="ob")
            nc.vector.tensor_copy(out=ob, in_=ps[0:1, :])
            nc.sync.dma_start(out=logits_d[0:1, c * 512:(c + 1) * 512], in_=ob)

    nc.compile()
    return nc


_prog = None


def _get_prog():
    global _prog
    if _prog is None:
        _prog = _build_program()
    return _prog


def _make_in_maps(inputs):
    dev = _prep_weights(inputs)
    idx = np.asarray(inputs["idx"])
    wte = np.asarray(inputs["wte"], np.float32)
    wpe = np.asarray(inputs["wpe"], np.float32)
    x0 = wte[idx] + wpe[None, :, :]                     # [8,1024,768] fp32
    x0 = np.ascontiguousarray(
        x0.transpose(0, 2, 1).reshape(B, KT, 128, T).transpose(0, 2, 1, 3)
    )                                                   # [8,128,6,1024]
    in_maps = []
    for b in range(B):
        m = {"x0": x0[b]}
        m.update(dev)
        in_maps.append(m)
    return in_maps


def _assemble(results, inputs):
    lm_b = np.asarray(inputs["lm_b"], np.float32)
    out = np.stack([results[b]["logits"][0, :V] for b in range(B)])
    out = out + lm_b[None, :]
    return out.reshape(B, 1, V).astype(np.float32)


def kernel(**inputs):
    in_maps = _make_in_maps(inputs)
    nc = _get_prog()
    res = run_bass_kernel_spmd(nc, in_maps, list(range(B)))
    return _assemble(res.results, inputs)


# revision 22
# speedup vs baseline: 1.0132x; 1.0132x over previous
"""GPT-2-small forward on 8 trn2 NeuronCores, data-parallel over batch.

Layout convention on device: activations are stored transposed,
[feature -> partitions (tiles of 128), token -> free dim].  Every matmul is
out = lhsT.T @ rhs with the contraction dim on partitions, so no transposes
are ever needed on device:
  - k/q:    lhsT = W.T tile (stationary), rhs = h_T        -> out [feat, tok]
  - v:      lhsT = h_T tile (stationary), rhs = W.T slice  -> out [tok, feat]
  - scores: lhsT = k_T tile, rhs = q_T                     -> S_T [tk, tq]
  - y:      lhsT = v tile (with a fused ones column so the softmax
            denominator falls out as psum row 64), rhs = exp(S_T)
Softmax reductions over tk (partitions) ride the tensor engine; LN stats do
too (ones-vector matmuls).  Residual stream stays fp32 in SBUF the whole
forward; matmul operands are bf16.  The last layer only computes
q/attention/proj/MLP for the final token (that is all the lm head reads).
"""

import sys

sys.path.insert(0, "/opt/trn_rl_repo")

from contextlib import ExitStack

import ml_dtypes
import numpy as np

import concourse.bass as bass
import concourse.tile as tile
from concourse import bacc, mybir
from concourse.bass_utils import run_bass_kernel_spmd

BF16NP = ml_dtypes.bfloat16
F32 = mybir.dt.float32
BF = mybir.dt.bfloat16

B, T, E, H, L, V = 8, 1024, 768, 12, 12, 50304
KT = E // 128          # 6 k-tiles of the embedding dim
D = E // H             # 64 head dim
F = 4 * E              # 3072
FT = F // 128          # 24
NVC = (V + 511) // 512  # 99 wte chunks of 512 vocab cols
VP = NVC * 512          # 50688 padded vocab
TS_FULL = [(0, 512), (512, 512)]
TS_LAST = [(1023, 1)]


# --------------------------------------------------------------------------
# host-side weight prep (cached across kernel() calls)
# --------------------------------------------------------------------------

def _lhsT(w):
    """[O, I] fp32 -> [128, I//128, O] bf16 (lhsT-ready, partition major)."""
    I = w.shape[1]
    return np.ascontiguousarray(
        w.T.reshape(I // 128, 128, -1).transpose(1, 0, 2).astype(BF16NP)
    )


def _pcol(b, nt):
    """per-feature vector [nt*128] -> [128, nt] (partition major)."""
    return np.ascontiguousarray(np.asarray(b, np.float32).reshape(nt, 128).T)


_prep_cache = {}


def _prep_weights(inp):
    key = (id(inp["wte"]), id(inp["qkv_w"]))
    if key in _prep_cache:
        return _prep_cache[key][1]

    qkvw = np.stack([_lhsT(inp["qkv_w"][l]) for l in range(L)])        # [L,128,6,2304]
    qkvw = np.ascontiguousarray(
        qkvw.reshape(L, 128, KT, 3, 768).transpose(0, 3, 1, 2, 4)
    )                                                                   # [L,3,128,6,768]
    projw = np.stack([_lhsT(inp["proj_w"][l]) for l in range(L)])       # [L,128,6,768]
    c1w = np.stack([_lhsT(inp["c1_w"][l]) for l in range(L)])           # [L,128,6,3072]
    c1w = np.ascontiguousarray(
        c1w.reshape(L, 128, KT, 4, 768).transpose(0, 3, 1, 2, 4)
    )                                                                   # [L,4,128,6,768]
    c2w = np.stack([_lhsT(inp["c2_w"][l]) for l in range(L)])           # [L,128,24,768]
    c2w = np.ascontiguousarray(
        c2w.reshape(L, 128, FT, 6, 128).transpose(0, 3, 1, 2, 4)
    )                                                                   # [L,6,128,24,128]

    wteT = np.asarray(inp["wte"], np.float32).T.astype(BF16NP)          # [768, V]
    wteT = np.concatenate([wteT, np.zeros((E, VP - V), BF16NP)], axis=1)
    wteT = np.ascontiguousarray(
        wteT.reshape(KT, 128, NVC, 512).transpose(2, 1, 0, 3)
    )                                                                   # [99,128,6,512]

    qkv_b = np.asarray(inp["qkv_b"], np.float32)
    dev = dict(
        qkvw=qkvw, projw=projw, c1w=c1w, c2w=c2w, wteT=wteT,
        kqb=np.stack([_pcol(qkv_b[l, :1536], 12) for l in range(L)]),
        vb=np.ascontiguousarray(qkv_b[:, 1536:]),                       # [L,768]
        projb=np.stack([_pcol(inp["proj_b"][l], 6) for l in range(L)]),
        c1b=np.stack([_pcol(inp["c1_b"][l], 24) for l in range(L)]),
        c2b=np.stack([_pcol(inp["c2_b"][l], 6) for l in range(L)]),
        lnw=np.stack([_pcol(inp["ln_w"][l], 6) for l in range(L)]),
        lnb=np.stack([_pcol(inp["ln_b"][l], 6) for l in range(L)]),
        lnfw=_pcol(inp["lnf_w"], 6),
        lnfb=_pcol(inp["lnf_b"], 6),
    )
    _prep_cache.clear()
    _prep_cache[key] = ((inp["wte"], inp["qkv_w"]), dev)  # hold refs so ids stay valid
    return dev


# --------------------------------------------------------------------------
# device program
# --------------------------------------------------------------------------

def _emit_ln(nc, pools, x_T, w_sb, kt_w, b_sb, kt_b, h_bf, slices, eps_sb, ones_bf,
             onesf=None):
    """h_bf[:, :, sl] = LN(x_T[:, :, sl]) * w + b   (bf16 out)."""
    A = mybir.AluOpType
    for (t0, tn) in slices:
        ssum = pools["sp"].tile([1, 512], F32, tag="sp", name="ssum")
        qsum = pools["sp"].tile([1, 512], F32, tag="sp", name="qsum")
        for kt in range(KT):
            xbt = pools["xb"].tile([128, 512], BF, tag="xb", name="xbt")
            nc.vector.tensor_copy(out=xbt[:, :tn], in_=x_T[:, kt, t0:t0 + tn])
            sqt = pools["xb"].tile([128, 512], BF, tag="sq", name="sqt")
            nc.vector.tensor_mul(sqt[:, :tn], xbt[:, :tn], xbt[:, :tn])
            nc.tensor.matmul(ssum[0:1, :tn], ones_bf[:, 0:1], xbt[:, :tn],
                             start=(kt == 0), stop=(kt == KT - 1))
            nc.tensor.matmul(qsum[0:1, :tn], ones_bf[:, 0:1], sqt[:, :tn],
                             start=(kt == 0), stop=(kt == KT - 1))
        mean = pools["stat"].tile([1, 512], F32, tag="stat", name="mean")
        nc.scalar.activation(mean[:, :tn], ssum[0:1, :tn],
                             mybir.ActivationFunctionType.Copy, scale=1.0 / E)
        ex2 = pools["stat"].tile([1, 512], F32, tag="stat", name="ex2")
        nc.scalar.activation(ex2[:, :tn], qsum[0:1, :tn],
                             mybir.ActivationFunctionType.Copy, scale=1.0 / E)
        var = pools["stat"].tile([1, 512], F32, tag="stat", name="var")
        nc.vector.scalar_tensor_tensor(
            out=var[:, :tn], in0=mean[:, :tn], scalar=-1.0, in1=mean[:, :tn],
            op0=A.mult, op1=A.mult)                                      # -mean^2
        nc.vector.tensor_add(var[:, :tn], var[:, :tn], ex2[:, :tn])
        sdv = pools["stat"].tile([1, 512], F32, tag="stat", name="sdv")
        nc.scalar.activation(sdv[:, :tn], var[:, :tn],
                             mybir.ActivationFunctionType.Sqrt,
                             bias=eps_sb[0:1, 0:1])
        rstd = pools["stat"].tile([1, 512], F32, tag="stat", name="rstd")
        nc.vector.reciprocal(rstd[:, :tn], sdv[:, :tn])
        # broadcast across partitions via K=1 outer product with a ones column
        mbc = pools["bcp"].tile([128, 512], F32, tag="bc", name="mbc")
        nc.tensor.matmul(mbc[:, :tn], onesf[0:1, :], mean[0:1, :tn],
                         start=True, stop=True)
        rbc = pools["bcp"].tile([128, 512], F32, tag="bc", name="rbc")
        nc.tensor.matmul(rbc[:, :tn], onesf[0:1, :], rstd[0:1, :tn],
                         start=True, stop=True)
        for kt in range(KT):
            tt = pools["tmp"].tile([128, 512], F32, tag="tmp", name="tt")
            nc.vector.tensor_sub(tt[:, :tn], x_T[:, kt, t0:t0 + tn], mbc[:, :tn])
            nc.vector.scalar_tensor_tensor(
                out=tt[:, :tn], in0=tt[:, :tn],
                scalar=w_sb[:, kt_w + kt:kt_w + kt + 1],
                in1=rbc[:, :tn], op0=A.mult, op1=A.mult)
            nc.vector.tensor_scalar_add(
                h_bf[:, kt, t0:t0 + tn], tt[:, :tn],
                b_sb[:, kt_b + kt:kt_b + kt + 1])


def _emit_layer(nc, pools, dram, x_T, eps_sb, ones_bf, masks, l, tq_slices,
                onesf=None):
    pp, wp, sm, act = pools["pp"], pools["w"], pools["sm"], pools["act"]
    A = mybir.AluOpType

    # per-layer small params
    lnp = sm.tile([128, 12], F32, tag="lnp", name="lnp")   # cols 0:6 w, 6:12 b
    nc.sync.dma_start(out=lnp[:, 0:6], in_=dram["lnw"][l])
    nc.sync.dma_start(out=lnp[:, 6:12], in_=dram["lnb"][l])
    kqb = sm.tile([128, 12], F32, tag="kqb", name="kqb")
    nc.sync.dma_start(out=kqb, in_=dram["kqb"][l])
    pcb = sm.tile([128, 12], F32, tag="pcb", name="pcb")   # 0:6 projb, 6:12 c2b
    nc.sync.dma_start(out=pcb[:, 0:6], in_=dram["projb"][l])
    nc.sync.dma_start(out=pcb[:, 6:12], in_=dram["c2b"][l])
    c1b = sm.tile([128, 24], F32, tag="c1b", name="c1b")
    nc.sync.dma_start(out=c1b, in_=dram["c1b"][l])
    vbc = sm.tile([128, 768], F32, tag="vbc", bufs=1, name="vbc")
    nc.sync.dma_start(out=vbc, in_=dram["vb"][l:l + 1, :].to_broadcast([128, 768]))

    # ---------------- LN1 -> h (full T: k and v always need every token)
    h_bf = act.tile([128, KT, T], BF, tag="h", bufs=1, name="h_bf")
    _emit_ln(nc, pools, x_T, lnp, 0, lnp, 6, h_bf, TS_FULL, eps_sb, ones_bf, onesf)

    # ---------------- qkv
    k_T = act.tile([128, KT, T], BF, tag="k", bufs=1, name="k_T")
    q_T = act.tile([128, KT, T], BF, tag="q", bufs=1, name="q_T")
    for ci, dst, bo, slices in ((0, k_T, 0, TS_FULL), (1, q_T, 6, tq_slices)):
        w_t = wp.tile([128, KT, 768], BF, tag="w", name="wkq")
        nc.sync.dma_start(out=w_t, in_=dram["qkvw"][l, ci])
        for o in range(6):
            for (t0, tn) in slices:
                ps = pp.tile([128, 512], F32, tag="pp", name="ps_kq")
                for kt in range(KT):
                    nc.tensor.matmul(ps[:, :tn], w_t[:, kt, o * 128:(o + 1) * 128],
                                     h_bf[:, kt, t0:t0 + tn],
                                     start=(kt == 0), stop=(kt == KT - 1))
                nc.vector.tensor_scalar_add(
                    dst[:, o, t0:t0 + tn], ps[:, :tn], kqb[:, bo + o:bo + o + 1])

    # v in [token, head, d] layout
    wv = wp.tile([128, KT, 768], BF, tag="w", name="wv")
    nc.sync.dma_start(out=wv, in_=dram["qkvw"][l, 2])
    v_sb = act.tile([128, 8, H, 64], BF, tag="v", bufs=1, name="v_sb")
    for tt in range(8):
        for (o0, on, h0, hn) in ((0, 512, 0, 8), (512, 256, 8, 4)):
            ps = pp.tile([128, 512], F32, tag="pp", name="ps_v")
            for kt in range(KT):
                nc.tensor.matmul(ps[:, :on], h_bf[:, kt, tt * 128:(tt + 1) * 128],
                                 wv[:, kt, o0:o0 + on],
                                 start=(kt == 0), stop=(kt == KT - 1))
            nc.vector.tensor_add(
                v_sb[:, tt, h0:h0 + hn, :],
                ps[:, :on].rearrange("p (h d) -> p h d", d=64),
                vbc[:, o0:o0 + on].rearrange("p (h d) -> p h d", d=64))

    # ---------------- attention + proj (per tq slice)
    wpj = wp.tile([128, KT, 768], BF, tag="w", name="wpj")
    nc.sync.dma_start(out=wpj, in_=dram["projw"][l])
    for (t0, tn) in tq_slices:
        rmax = (t0 + tn - 1) // 128
        y_T = act.tile([128, KT, 512], BF, tag="y", bufs=1, name="y_T")
        for hp in range(6):
            # streamed attention: per r-block, scores -> exp -> accumulate
            # denominator + y.  exp tiles are tiny and triple-buffered so PE
            # (scores/den/y) and ACT (exp) pipeline across r.
            den_a = pools["sp"].tile([1, 512], F32, tag="sp", name="den_a")
            den_b = pools["sp"].tile([1, 512], F32, tag="sp", name="den_b")
            dens = [den_a, den_b]
            py = pp.tile([128, 512], F32, tag="pp", name="ps_y")
            for r in range(rmax + 1):
                m = r - (t0 // 128)  # mask variant; >=0 only on diagonal blocks
                exs = []
                for half in (0, 1):
                    p0 = 64 * half
                    ps = pp.tile([128, 512], F32, tag="pp", name="ps_s")
                    nc.tensor.matmul(
                        ps[:, :tn],
                        k_T[p0:p0 + 64, hp, r * 128:(r + 1) * 128],
                        q_T[p0:p0 + 64, hp, t0:t0 + tn],
                        start=True, stop=True)
                    ex = act.tile([128, 512], BF, tag="ex", bufs=4, name="ex")
                    nc.scalar.activation(ex[:, :tn], ps[:, :tn],
                                         mybir.ActivationFunctionType.Exp)
                    if tn > 1 and m >= 0:
                        nc.vector.tensor_mul(ex[:, :tn], ex[:, :tn],
                                             masks[:, m, :tn])
                    exs.append(ex)
                for half, ex in ((0, exs[0]), (1, exs[1])):
                    nc.tensor.matmul(dens[half][0:1, :tn], ones_bf[:, 0:1],
                                     ex[:, :tn],
                                     start=(r == 0), stop=(r == rmax))
                nc.tensor.matmul(py[0:64, :tn], v_sb[:, r, 2 * hp, :],
                                 exs[0][:, :tn], start=(r == 0), stop=(r == rmax))
                nc.tensor.matmul(py[64:128, :tn], v_sb[:, r, 2 * hp + 1, :],
                                 exs[1][:, :tn], start=(r == 0), stop=(r == rmax),
                                 tile_position=(0, 64))
            bca = pools["bcp"].tile([128, 512], F32, tag="bc", name="bca")
            for half in (0, 1):
                rcp = pools["rcp"].tile([1, 512], F32, tag="rcp", name="rcp")
                nc.vector.reciprocal(rcp[:, :tn], dens[half][0:1, :tn])
                nc.tensor.matmul(bca[64 * half:64 * half + 64, :tn],
                                 onesf[0:1, 0:64], rcp[0:1, :tn],
                                 start=True, stop=True,
                                 tile_position=(0, 64 * half))
            bcs = pools["xb"].tile([128, 512], F32, tag="bcs", name="bcs")
            nc.scalar.activation(bcs[:, :tn], bca[:, :tn],
                                 mybir.ActivationFunctionType.Copy)
            nc.vector.tensor_mul(y_T[0:64, hp, :tn], py[0:64, :tn],
                                 bcs[0:64, :tn])
            nc.vector.tensor_mul(y_T[64:128, hp, :tn], py[64:128, :tn],
                                 bcs[64:128, :tn])

        for o in range(6):  # proj + residual
            ps = pp.tile([128, 512], F32, tag="pp", name="ps_p")
            for kt in range(KT):
                nc.tensor.matmul(ps[:, :tn], wpj[:, kt, o * 128:(o + 1) * 128],
                                 y_T[:, kt, :tn],
                                 start=(kt == 0), stop=(kt == KT - 1))
            nc.vector.scalar_tensor_tensor(
                out=x_T[:, o, t0:t0 + tn], in0=ps[:, :tn],
                scalar=pcb[:, o:o + 1], in1=x_T[:, o, t0:t0 + tn],
                op0=A.add, op1=A.add)

    # ---------------- LN2 + MLP (slice-major; c1/c2 weights re-streamed per
    # slice to keep only one mid buffer live -- DMA is far from the bottleneck)
    h2 = act.tile([128, KT, T], BF, tag="h", bufs=1, name="h2")
    _emit_ln(nc, pools, x_T, lnp, 0, lnp, 6, h2, tq_slices, eps_sb, ones_bf, onesf)
    for (t0, tn) in tq_slices:
        mid = pools["mid"].tile([128, FT, 512], BF, tag="mid", name="mid")
        for ci in range(4):
            wc = wp.tile([128, KT, 768], BF, tag="w", name="wc1")
            nc.sync.dma_start(out=wc, in_=dram["c1w"][l, ci])
            for oo in range(6):
                o = ci * 6 + oo
                ps = pp.tile([128, 512], F32, tag="pp", name="ps_c1")
                for kt in range(KT):
                    nc.tensor.matmul(ps[:, :tn], wc[:, kt, oo * 128:(oo + 1) * 128],
                                     h2[:, kt, t0:t0 + tn],
                                     start=(kt == 0), stop=(kt == KT - 1))
                nc.scalar.activation(mid[:, o, :tn], ps[:, :tn],
                                     mybir.ActivationFunctionType.Gelu,
                                     bias=c1b[:, o:o + 1])
        for o in range(6):
            wc2 = wp.tile([128, FT, 128], BF, tag="w", name="wc2")
            nc.sync.dma_start(out=wc2, in_=dram["c2w"][l, o])
            ps = pp.tile([128, 512], F32, tag="pp", name="ps_c2")
            for kt in range(FT):
                nc.tensor.matmul(ps[:, :tn], wc2[:, kt, :], mid[:, kt, :tn],
                                 start=(kt == 0), stop=(kt == FT - 1))
            nc.vector.scalar_tensor_tensor(
                out=x_T[:, o, t0:t0 + tn], in0=ps[:, :tn],
                scalar=pcb[:, 6 + o:7 + o], in1=x_T[:, o, t0:t0 + tn],
                op0=A.add, op1=A.add)


def _build_program():
    nc = bacc.Bacc("TRN2", target_bir_lowering=False, debug=False,
                   enable_asserts=False, num_devices=8)
    d = {}
    d["x0"] = nc.dram_tensor("x0", [128, KT, T], F32, kind="ExternalInput").ap()
    d["qkvw"] = nc.dram_tensor("qkvw", [L, 3, 128, KT, 768], BF, kind="ExternalInput").ap()
    d["projw"] = nc.dram_tensor("projw", [L, 128, KT, 768], BF, kind="ExternalInput").ap()
    d["c1w"] = nc.dram_tensor("c1w", [L, 4, 128, KT, 768], BF, kind="ExternalInput").ap()
    d["c2w"] = nc.dram_tensor("c2w", [L, 6, 128, FT, 128], BF, kind="ExternalInput").ap()
    d["wteT"] = nc.dram_tensor("wteT", [NVC, 128, KT, 512], BF, kind="ExternalInput").ap()
    for nm, shp in (("kqb", [L, 128, 12]), ("vb", [L, 768]),
                    ("projb", [L, 128, 6]), ("c1b", [L, 128, 24]),
                    ("c2b", [L, 128, 6]), ("lnw", [L, 128, 6]),
                    ("lnb", [L, 128, 6]), ("lnfw", [128, 6]), ("lnfb", [128, 6])):
        d[nm] = nc.dram_tensor(nm, shp, F32, kind="ExternalInput").ap()
    logits_d = nc.dram_tensor("logits", [1, VP], F32, kind="ExternalOutput").ap()

    with tile.TileContext(nc) as tc, ExitStack() as ctx:
        pools = {
            "pp": ctx.enter_context(tc.tile_pool(name="pp", bufs=4, space="PSUM")),
            "sp": ctx.enter_context(tc.tile_pool(name="sp", bufs=2, space="PSUM")),
            "bcp": ctx.enter_context(tc.tile_pool(name="bcp", bufs=2, space="PSUM")),
            "w": ctx.enter_context(tc.tile_pool(name="w", bufs=4)),
            "act": ctx.enter_context(tc.tile_pool(name="act", bufs=2)),
            "mid": ctx.enter_context(tc.tile_pool(name="mid", bufs=1)),
            "xb": ctx.enter_context(tc.tile_pool(name="xb", bufs=2)),
            "stat": ctx.enter_context(tc.tile_pool(name="stat", bufs=3)),
            "rcp": ctx.enter_context(tc.tile_pool(name="rcp", bufs=2)),
            "tmp": ctx.enter_context(tc.tile_pool(name="tmp", bufs=2)),
            "sm": ctx.enter_context(tc.tile_pool(name="sm", bufs=2)),
            "const": ctx.enter_context(tc.tile_pool(name="const", bufs=1)),
        }
        cp = pools["const"]
        x_T = cp.tile([128, KT, T], F32, tag="x", name="x_T")
        nc.sync.dma_start(out=x_T, in_=d["x0"])
        ones_bf = cp.tile([128, 1], BF, tag="ones", name="ones_bf")
        nc.vector.memset(ones_bf, 1.0)
        onesf = cp.tile([65, 128], F32, tag="onesf", name="onesf")
        nc.vector.memset(onesf, 1.0)
        eps_sb = cp.tile([1, 1], F32, tag="eps", name="eps_sb")
        nc.vector.memset(eps_sb, 1e-5)
        masks = cp.tile([128, 4, 512], BF, tag="masks", name="masks")
        nc.vector.memset(masks, 1.0)
        for m in range(4):
            # keep 1.0 iff tq - tk >= 0, i.e. -x + y - 128*m >= 0
            nc.gpsimd.affine_select(
                out=masks[:, m, :], in_=masks[:, m, :],
                compare_op=mybir.AluOpType.is_ge, fill=0.0,
                base=-128 * m, pattern=[[1, 512]], channel_multiplier=-1)

        for l in range(L):
            _emit_layer(nc, pools, d, x_T, eps_sb, ones_bf, masks, l,
                        TS_FULL if l < L - 1 else TS_LAST, onesf)

        # final LN on last token only
        lnf = pools["sm"].tile([128, 12], F32, tag="lnf", name="lnf")
        nc.sync.dma_start(out=lnf[:, 0:6], in_=d["lnfw"])
        nc.sync.dma_start(out=lnf[:, 6:12], in_=d["lnfb"])
        hf = pools["act"].tile([128, KT, T], BF, tag="h", bufs=1, name="hf")
        _emit_ln(nc, pools, x_T, lnf, 0, lnf, 6, hf, TS_LAST, eps_sb, ones_bf, onesf)

        # lm head: [1,768] @ [768, 50688]
        for c in range(NVC):
            wt = pools["w"].tile([128, KT, 512], BF, tag="w", name="wt")
            nc.sync.dma_start(out=wt, in_=d["wteT"][c])
            ps = pools["sp"].tile([1, 512], F32, tag="sp", name="ps_lm")
            for kt in range(KT):
                nc.tensor.matmul(ps[0:1, :], hf[:, kt, 1023:1024], wt[:, kt, :],
                                 start=(kt == 0), stop=(kt == KT - 1))
            ob = pools["stat"].tile([1, 512], F32, tag="ob", bufs=2, name="ob")
            nc.vector.tensor_copy(out=ob, in_=ps[0:1, :])
            nc.sync.dma_start(out=logits_d[0:1, c * 512:(c + 1) * 512], in_=ob)

    nc.compile()
    return nc


_prog = None


def _get_prog():
    global _prog
    if _prog is None:
        _prog = _build_program()
    return _prog


def _make_in_maps(inputs):
    dev = _prep_weights(inputs)
    idx = np.asarray(inputs["idx"])
    wte = np.asarray(inputs["wte"], np.float32)
    wpe = np.asarray(inputs["wpe"], np.float32)
    x0 = wte[idx] + wpe[None, :, :]                     # [8,1024,768] fp32
    x0 = np.ascontiguousarray(
        x0.transpose(0, 2, 1).reshape(B, KT, 128, T).transpose(0, 2, 1, 3)
    )                                                   # [8,128,6,1024]
    in_maps = []
    for b in range(B):
        m = {"x0": x0[b]}
        m.update(dev)
        in_maps.append(m)
    return in_maps


def _assemble(results, inputs):
    lm_b = np.asarray(inputs["lm_b"], np.float32)
    out = np.stack([results[b]["logits"][0, :V] for b in range(B)])
    out = out + lm_b[None, :]
    return out.reshape(B, 1, V).astype(np.float32)


def kernel(**inputs):
    in_maps = _make_in_maps(inputs)
    nc = _get_prog()
    res = run_bass_kernel_spmd(nc, in_maps, list(range(B)))
    return _assemble(res.results, inputs)


# revision 24
# speedup vs baseline: 18.7973x; 18.5528x over previous
"""GPT-2-small forward on 8 trn2 NeuronCores, data-parallel over batch.

Layout convention on device: activations are stored transposed,
[feature -> partitions (tiles of 128), token -> free dim].  Every matmul is
out = lhsT.T @ rhs with the contraction dim on partitions, so no transposes
are ever needed on device:
  - k/q:    lhsT = W.T tile (stationary), rhs = h_T        -> out [feat, tok]
  - v:      lhsT = h_T tile (stationary), rhs = W.T slice  -> out [tok, feat]
  - scores: lhsT = k_T tile, rhs = q_T                     -> S_T [tk, tq]
  - y:      lhsT = v tile (with a fused ones column so the softmax
            denominator falls out as psum row 64), rhs = exp(S_T)
Softmax reductions over tk (partitions) ride the tensor engine; LN stats do
too (ones-vector matmuls).  Residual stream stays fp32 in SBUF the whole
forward; matmul operands are bf16.  The last layer only computes
q/attention/proj/MLP for the final token (that is all the lm head reads).
"""

import sys

sys.path.insert(0, "/opt/trn_rl_repo")

from contextlib import ExitStack

import ml_dtypes
import numpy as np

import concourse.bass as bass
import concourse.tile as tile
from concourse import bacc, mybir
from concourse.bass_utils import run_bass_kernel_spmd

BF16NP = ml_dtypes.bfloat16
F32 = mybir.dt.float32
BF = mybir.dt.bfloat16

B, T, E, H, L, V = 8, 1024, 768, 12, 12, 50304
KT = E // 128          # 6 k-tiles of the embedding dim
D = E // H             # 64 head dim
F = 4 * E              # 3072
FT = F // 128          # 24
NVC = (V + 511) // 512  # 99 wte chunks of 512 vocab cols
VP = NVC * 512          # 50688 padded vocab
TS_FULL = [(0, 512), (512, 512)]
TS_LAST = [(1023, 1)]


# --------------------------------------------------------------------------
# host-side weight prep (cached across kernel() calls)
# --------------------------------------------------------------------------

def _lhsT(w):
    """[O, I] fp32 -> [128, I//128, O] bf16 (lhsT-ready, partition major)."""
    I = w.shape[1]
    return np.ascontiguousarray(
        w.T.reshape(I // 128, 128, -1).transpose(1, 0, 2).astype(BF16NP)
    )


def _pcol(b, nt):
    """per-feature vector [nt*128] -> [128, nt] (partition major)."""
    return np.ascontiguousarray(np.asarray(b, np.float32).reshape(nt, 128).T)


_prep_cache = {}


def _prep_weights(inp):
    key = (id(inp["wte"]), id(inp["qkv_w"]))
    if key in _prep_cache:
        return _prep_cache[key][1]

    qkvw = np.stack([_lhsT(inp["qkv_w"][l]) for l in range(L)])        # [L,128,6,2304]
    qkvw = np.ascontiguousarray(
        qkvw.reshape(L, 128, KT, 3, 768).transpose(0, 3, 1, 2, 4)
    )                                                                   # [L,3,128,6,768]
    projw = np.stack([_lhsT(inp["proj_w"][l]) for l in range(L)])       # [L,128,6,768]
    c1w = np.stack([_lhsT(inp["c1_w"][l]) for l in range(L)])           # [L,128,6,3072]
    c1w = np.ascontiguousarray(
        c1w.reshape(L, 128, KT, 4, 768).transpose(0, 3, 1, 2, 4)
    )                                                                   # [L,4,128,6,768]
    c2w = np.stack([_lhsT(inp["c2_w"][l]) for l in range(L)])           # [L,128,24,768]
    c2w = np.ascontiguousarray(
        c2w.reshape(L, 128, FT, 6, 128).transpose(0, 3, 1, 2, 4)
    )                                                                   # [L,6,128,24,128]

    wteT = np.asarray(inp["wte"], np.float32).T.astype(BF16NP)          # [768, V]
    wteT = np.concatenate([wteT, np.zeros((E, VP - V), BF16NP)], axis=1)
    wteT = np.ascontiguousarray(
        wteT.reshape(KT, 128, NVC, 512).transpose(2, 1, 0, 3)
    )                                                                   # [99,128,6,512]

    qkv_b = np.asarray(inp["qkv_b"], np.float32)
    dev = dict(
        qkvw=qkvw, projw=projw, c1w=c1w, c2w=c2w, wteT=wteT,
        kqb=np.stack([_pcol(qkv_b[l, :1536], 12) for l in range(L)]),
        vb=np.ascontiguousarray(qkv_b[:, 1536:]),                       # [L,768]
        projb=np.stack([_pcol(inp["proj_b"][l], 6) for l in range(L)]),
        c1b=np.stack([_pcol(inp["c1_b"][l], 24) for l in range(L)]),
        c2b=np.stack([_pcol(inp["c2_b"][l], 6) for l in range(L)]),
        lnw=np.stack([_pcol(inp["ln_w"][l], 6) for l in range(L)]),
        lnb=np.stack([_pcol(inp["ln_b"][l], 6) for l in range(L)]),
        lnfw=_pcol(inp["lnf_w"], 6),
        lnfb=_pcol(inp["lnf_b"], 6),
    )
    _prep_cache.clear()
    _prep_cache[key] = ((inp["wte"], inp["qkv_w"]), dev)  # hold refs so ids stay valid
    return dev


# --------------------------------------------------------------------------
# device program
# --------------------------------------------------------------------------

def _emit_ln(nc, pools, x_T, w_sb, kt_w, b_sb, kt_b, h_bf, slices, eps_sb, ones_bf,
             onesf=None):
    """h_bf[:, :, sl] = LN(x_T[:, :, sl]) * w + b   (bf16 out)."""
    A = mybir.AluOpType
    A0 = mybir.AluOpType
    for (t0, tn) in slices:
        # accumulate sum(x) and sum(x^2) over the 6 feature tiles on DVE,
        # then one K=128 ones-matmul each for the cross-partition reduction
        xacc = pools["xb"].tile([128, 512], BF, tag="xa", name="xacc")
        qacc = pools["xb"].tile([128, 512], BF, tag="qa", name="qacc")
        for kt in range(KT):
            xbt = pools["xb"].tile([128, 512], BF, tag="xb", name="xbt")
            nc.vector.tensor_copy(out=xbt[:, :tn], in_=x_T[:, kt, t0:t0 + tn])
            if kt == 0:
                nc.vector.tensor_mul(qacc[:, :tn], xbt[:, :tn], xbt[:, :tn])
            else:
                nc.vector.tensor_add(xacc[:, :tn], xacc[:, :tn], xbt[:, :tn])
                sqt = pools["xb"].tile([128, 512], BF, tag="sq", name="sqt")
                nc.vector.tensor_mul(sqt[:, :tn], xbt[:, :tn], xbt[:, :tn])
                nc.vector.tensor_add(qacc[:, :tn], qacc[:, :tn], sqt[:, :tn])
            if kt == 0:
                nc.vector.tensor_copy(out=xacc[:, :tn], in_=xbt[:, :tn])
        ssum = pools["sp"].tile([1, 512], F32, tag="sp", name="ssum")
        qsum = pools["sp"].tile([1, 512], F32, tag="sp", name="qsum")
        nc.tensor.matmul(ssum[0:1, :tn], ones_bf[:, 0:1], xacc[:, :tn],
                         start=True, stop=True)
        nc.tensor.matmul(qsum[0:1, :tn], ones_bf[:, 0:1], qacc[:, :tn],
                         start=True, stop=True)
        mean = pools["stat"].tile([1, 512], F32, tag="stat", name="mean")
        nc.scalar.activation(mean[:, :tn], ssum[0:1, :tn],
                             mybir.ActivationFunctionType.Copy, scale=1.0 / E)
        ex2 = pools["stat"].tile([1, 512], F32, tag="stat", name="ex2")
        nc.scalar.activation(ex2[:, :tn], qsum[0:1, :tn],
                             mybir.ActivationFunctionType.Copy, scale=1.0 / E)
        var = pools["stat"].tile([1, 512], F32, tag="stat", name="var")
        nc.vector.scalar_tensor_tensor(
            out=var[:, :tn], in0=mean[:, :tn], scalar=-1.0, in1=mean[:, :tn],
            op0=A.mult, op1=A.mult)                                      # -mean^2
        nc.vector.tensor_add(var[:, :tn], var[:, :tn], ex2[:, :tn])
        sdv = pools["stat"].tile([1, 512], F32, tag="stat", name="sdv")
        nc.scalar.activation(sdv[:, :tn], var[:, :tn],
                             mybir.ActivationFunctionType.Sqrt,
                             bias=eps_sb[0:1, 0:1])
        rstd = pools["stat"].tile([1, 512], F32, tag="stat", name="rstd")
        nc.vector.reciprocal(rstd[:, :tn], sdv[:, :tn])
        # broadcast across partitions via K=1 outer product with a ones column
        mbc = pools["bcp"].tile([128, 512], F32, tag="bc", name="mbc")
        nc.tensor.matmul(mbc[:, :tn], onesf[0:1, :], mean[0:1, :tn],
                         start=True, stop=True)
        rbc = pools["bcp"].tile([128, 512], F32, tag="bc", name="rbc")
        nc.tensor.matmul(rbc[:, :tn], onesf[0:1, :], rstd[0:1, :tn],
                         start=True, stop=True)
        for kt in range(KT):
            tt = pools["tmp"].tile([128, 512], F32, tag="tmp", name="tt")
            nc.vector.tensor_sub(tt[:, :tn], x_T[:, kt, t0:t0 + tn], mbc[:, :tn])
            nc.vector.scalar_tensor_tensor(
                out=tt[:, :tn], in0=tt[:, :tn],
                scalar=w_sb[:, kt_w + kt:kt_w + kt + 1],
                in1=rbc[:, :tn], op0=A.mult, op1=A.mult)
            nc.vector.tensor_scalar_add(
                h_bf[:, kt, t0:t0 + tn], tt[:, :tn],
                b_sb[:, kt_b + kt:kt_b + kt + 1])


def _emit_layer(nc, pools, dram, x_T, eps_sb, ones_bf, masks, l, tq_slices,
                onesf=None):
    pp, wp, sm, act = pools["pp"], pools["w"], pools["sm"], pools["act"]
    A = mybir.AluOpType

    # per-layer small params
    lnp = sm.tile([128, 12], F32, tag="lnp", name="lnp")   # cols 0:6 w, 6:12 b
    nc.sync.dma_start(out=lnp[:, 0:6], in_=dram["lnw"][l])
    nc.sync.dma_start(out=lnp[:, 6:12], in_=dram["lnb"][l])
    kqb = sm.tile([128, 12], F32, tag="kqb", name="kqb")
    nc.sync.dma_start(out=kqb, in_=dram["kqb"][l])
    pcb = sm.tile([128, 12], F32, tag="pcb", name="pcb")   # 0:6 projb, 6:12 c2b
    nc.sync.dma_start(out=pcb[:, 0:6], in_=dram["projb"][l])
    nc.sync.dma_start(out=pcb[:, 6:12], in_=dram["c2b"][l])
    c1b = sm.tile([128, 24], F32, tag="c1b", name="c1b")
    nc.sync.dma_start(out=c1b, in_=dram["c1b"][l])
    vbc = sm.tile([128, 768], F32, tag="vbc", bufs=1, name="vbc")
    nc.sync.dma_start(out=vbc, in_=dram["vb"][l:l + 1, :].to_broadcast([128, 768]))

    # ---------------- LN1 -> h (full T: k and v always need every token)
    h_bf = act.tile([128, KT, T], BF, tag="h", bufs=1, name="h_bf")
    _emit_ln(nc, pools, x_T, lnp, 0, lnp, 6, h_bf, TS_FULL, eps_sb, ones_bf, onesf)

    # ---------------- qkv
    k_T = act.tile([128, KT, T], BF, tag="k", bufs=1, name="k_T")
    q_T = act.tile([128, KT, T], BF, tag="q", bufs=1, name="q_T")
    for ci, dst, bo, slices in ((0, k_T, 0, TS_FULL), (1, q_T, 6, tq_slices)):
        w_t = wp.tile([128, KT, 768], BF, tag="w", name="wkq")
        nc.sync.dma_start(out=w_t, in_=dram["qkvw"][l, ci])
        for o in range(6):
            for (t0, tn) in slices:
                ps = pp.tile([128, 512], F32, tag="pp", name="ps_kq")
                for kt in range(KT):
                    nc.tensor.matmul(ps[:, :tn], w_t[:, kt, o * 128:(o + 1) * 128],
                                     h_bf[:, kt, t0:t0 + tn],
                                     start=(kt == 0), stop=(kt == KT - 1))
                nc.vector.tensor_scalar_add(
                    dst[:, o, t0:t0 + tn], ps[:, :tn], kqb[:, bo + o:bo + o + 1])

    # v in [token, head, d] layout
    wv = wp.tile([128, KT, 768], BF, tag="w", name="wv")
    nc.sync.dma_start(out=wv, in_=dram["qkvw"][l, 2])
    v_sb = act.tile([128, 8, H, 64], BF, tag="v", bufs=1, name="v_sb")
    for tt in range(8):
        for (o0, on, h0, hn) in ((0, 512, 0, 8), (512, 256, 8, 4)):
            ps = pp.tile([128, 512], F32, tag="pp", name="ps_v")
            for kt in range(KT):
                nc.tensor.matmul(ps[:, :on], h_bf[:, kt, tt * 128:(tt + 1) * 128],
                                 wv[:, kt, o0:o0 + on],
                                 start=(kt == 0), stop=(kt == KT - 1))
            nc.vector.tensor_add(
                v_sb[:, tt, h0:h0 + hn, :],
                ps[:, :on].rearrange("p (h d) -> p h d", d=64),
                vbc[:, o0:o0 + on].rearrange("p (h d) -> p h d", d=64))

    # ---------------- attention + proj (per tq slice)
    wpj = wp.tile([128, KT, 768], BF, tag="w", name="wpj")
    nc.sync.dma_start(out=wpj, in_=dram["projw"][l])
    for (t0, tn) in tq_slices:
        rmax = (t0 + tn - 1) // 128
        y_T = act.tile([128, KT, 512], BF, tag="y", bufs=1, name="y_T")
        for hp in range(6):
            # streamed attention: per r-block, scores -> exp -> accumulate
            # denominator + y.  exp tiles are tiny and triple-buffered so PE
            # (scores/den/y) and ACT (exp) pipeline across r.
            esums = [act.tile([128, 512], BF, tag="es", bufs=4, name="esum")
                     for _ in (0, 1)]
            py = pp.tile([128, 512], F32, tag="pp", name="ps_y")
            for r in range(rmax + 1):
                m = r - (t0 // 128)  # mask variant; >=0 only on diagonal blocks
                exs = []
                for half in (0, 1):
                    p0 = 64 * half
                    ps = pp.tile([128, 512], F32, tag="pp", name="ps_s")
                    nc.tensor.matmul(
                        ps[:, :tn],
                        k_T[p0:p0 + 64, hp, r * 128:(r + 1) * 128],
                        q_T[p0:p0 + 64, hp, t0:t0 + tn],
                        start=True, stop=True)
                    ex = act.tile([128, 512], BF, tag="ex", bufs=4, name="ex")
                    nc.scalar.activation(ex[:, :tn], ps[:, :tn],
                                         mybir.ActivationFunctionType.Exp)
                    if tn > 1 and m >= 0:
                        nc.vector.tensor_mul(ex[:, :tn], ex[:, :tn],
                                             masks[:, m, :tn])
                    # running exp-sum for the softmax denominator (DVE)
                    if r == 0:
                        nc.vector.tensor_copy(out=esums[half][:, :tn],
                                              in_=ex[:, :tn])
                    else:
                        nc.vector.tensor_add(esums[half][:, :tn],
                                             esums[half][:, :tn], ex[:, :tn])
                    exs.append(ex)
                nc.tensor.matmul(py[0:64, :tn], v_sb[:, r, 2 * hp, :],
                                 exs[0][:, :tn], start=(r == 0), stop=(r == rmax))
                nc.tensor.matmul(py[64:128, :tn], v_sb[:, r, 2 * hp + 1, :],
                                 exs[1][:, :tn], start=(r == 0), stop=(r == rmax),
                                 tile_position=(0, 64))
            bca = pools["bcp"].tile([128, 512], F32, tag="bc", name="bca")
            for half in (0, 1):
                den = pools["sp"].tile([1, 512], F32, tag="sp", name="den")
                nc.tensor.matmul(den[0:1, :tn], ones_bf[:, 0:1],
                                 esums[half][:, :tn], start=True, stop=True)
                rcp = pools["rcp"].tile([1, 512], F32, tag="rcp", name="rcp")
                nc.vector.reciprocal(rcp[:, :tn], den[0:1, :tn])
                nc.tensor.matmul(bca[64 * half:64 * half + 64, :tn],
                                 onesf[0:1, 0:64], rcp[0:1, :tn],
                                 start=True, stop=True,
                                 tile_position=(0, 64 * half))
            bcs = pools["xb"].tile([128, 512], F32, tag="bcs", name="bcs")
            nc.scalar.activation(bcs[:, :tn], bca[:, :tn],
                                 mybir.ActivationFunctionType.Copy)
            nc.vector.tensor_mul(y_T[0:64, hp, :tn], py[0:64, :tn],
                                 bcs[0:64, :tn])
            nc.vector.tensor_mul(y_T[64:128, hp, :tn], py[64:128, :tn],
                                 bcs[64:128, :tn])

        for o in range(6):  # proj + residual
            ps = pp.tile([128, 512], F32, tag="pp", name="ps_p")
            for kt in range(KT):
                nc.tensor.matmul(ps[:, :tn], wpj[:, kt, o * 128:(o + 1) * 128],
                                 y_T[:, kt, :tn],
                                 start=(kt == 0), stop=(kt == KT - 1))
            nc.vector.scalar_tensor_tensor(
                out=x_T[:, o, t0:t0 + tn], in0=ps[:, :tn],
                scalar=pcb[:, o:o + 1], in1=x_T[:, o, t0:t0 + tn],
                op0=A.add, op1=A.add)

    # ---------------- LN2 + MLP (slice-major; c1/c2 weights re-streamed per
    # slice to keep only one mid buffer live -- DMA is far from the bottleneck)
    h2 = act.tile([128, KT, T], BF, tag="h", bufs=1, name="h2")
    _emit_ln(nc, pools, x_T, lnp, 0, lnp, 6, h2, tq_slices, eps_sb, ones_bf, onesf)
    for (t0, tn) in tq_slices:
        mid = pools["mid"].tile([128, FT, 512], BF, tag="mid", name="mid")
        for ci in range(4):
            wc = wp.tile([128, KT, 768], BF, tag="w", name="wc1")
            nc.sync.dma_start(out=wc, in_=dram["c1w"][l, ci])
            for oo in range(6):
                o = ci * 6 + oo
                ps = pp.tile([128, 512], F32, tag="pp", name="ps_c1")
                for kt in range(KT):
                    nc.tensor.matmul(ps[:, :tn], wc[:, kt, oo * 128:(oo + 1) * 128],
                                     h2[:, kt, t0:t0 + tn],
                                     start=(kt == 0), stop=(kt == KT - 1))
                nc.scalar.activation(mid[:, o, :tn], ps[:, :tn],
                                     mybir.ActivationFunctionType.Gelu,
                                     bias=c1b[:, o:o + 1])
        for o in range(6):
            wc2 = wp.tile([128, FT, 128], BF, tag="w", name="wc2")
            nc.sync.dma_start(out=wc2, in_=dram["c2w"][l, o])
            ps = pp.tile([128, 512], F32, tag="pp", name="ps_c2")
            for kt in range(FT):
                nc.tensor.matmul(ps[:, :tn], wc2[:, kt, :], mid[:, kt, :tn],
                                 start=(kt == 0), stop=(kt == FT - 1))
            nc.vector.scalar_tensor_tensor(
                out=x_T[:, o, t0:t0 + tn], in0=ps[:, :tn],
                scalar=pcb[:, 6 + o:7 + o], in1=x_T[:, o, t0:t0 + tn],
                op0=A.add, op1=A.add)


def _build_program():
    nc = bacc.Bacc("TRN2", target_bir_lowering=False, debug=False,
                   enable_asserts=False, num_devices=8)
    d = {}
    d["x0"] = nc.dram_tensor("x0", [128, KT, T], F32, kind="ExternalInput").ap()
    d["qkvw"] = nc.dram_tensor("qkvw", [L, 3, 128, KT, 768], BF, kind="ExternalInput").ap()
    d["projw"] = nc.dram_tensor("projw", [L, 128, KT, 768], BF, kind="ExternalInput").ap()
    d["c1w"] = nc.dram_tensor("c1w", [L, 4, 128, KT, 768], BF, kind="ExternalInput").ap()
    d["c2w"] = nc.dram_tensor("c2w", [L, 6, 128, FT, 128], BF, kind="ExternalInput").ap()
    d["wteT"] = nc.dram_tensor("wteT", [NVC, 128, KT, 512], BF, kind="ExternalInput").ap()
    for nm, shp in (("kqb", [L, 128, 12]), ("vb", [L, 768]),
                    ("projb", [L, 128, 6]), ("c1b", [L, 128, 24]),
                    ("c2b", [L, 128, 6]), ("lnw", [L, 128, 6]),
                    ("lnb", [L, 128, 6]), ("lnfw", [128, 6]), ("lnfb", [128, 6])):
        d[nm] = nc.dram_tensor(nm, shp, F32, kind="ExternalInput").ap()
    logits_d = nc.dram_tensor("logits", [1, VP], F32, kind="ExternalOutput").ap()

    with tile.TileContext(nc) as tc, ExitStack() as ctx:
        pools = {
            "pp": ctx.enter_context(tc.tile_pool(name="pp", bufs=4, space="PSUM")),
            "sp": ctx.enter_context(tc.tile_pool(name="sp", bufs=2, space="PSUM")),
            "bcp": ctx.enter_context(tc.tile_pool(name="bcp", bufs=2, space="PSUM")),
            "w": ctx.enter_context(tc.tile_pool(name="w", bufs=4)),
            "act": ctx.enter_context(tc.tile_pool(name="act", bufs=2)),
            "mid": ctx.enter_context(tc.tile_pool(name="mid", bufs=1)),
            "xb": ctx.enter_context(tc.tile_pool(name="xb", bufs=2)),
            "stat": ctx.enter_context(tc.tile_pool(name="stat", bufs=3)),
            "rcp": ctx.enter_context(tc.tile_pool(name="rcp", bufs=2)),
            "tmp": ctx.enter_context(tc.tile_pool(name="tmp", bufs=2)),
            "sm": ctx.enter_context(tc.tile_pool(name="sm", bufs=2)),
            "const": ctx.enter_context(tc.tile_pool(name="const", bufs=1)),
        }
        cp = pools["const"]
        x_T = cp.tile([128, KT, T], F32, tag="x", name="x_T")
        nc.sync.dma_start(out=x_T, in_=d["x0"])
        ones_bf = cp.tile([128, 1], BF, tag="ones", name="ones_bf")
        nc.vector.memset(ones_bf, 1.0)
        onesf = cp.tile([65, 128], F32, tag="onesf", name="onesf")
        nc.vector.memset(onesf, 1.0)
        eps_sb = cp.tile([1, 1], F32, tag="eps", name="eps_sb")
        nc.vector.memset(eps_sb, 1e-5)
        masks = cp.tile([128, 4, 512], BF, tag="masks", name="masks")
        nc.vector.memset(masks, 1.0)
        for m in range(4):
            # keep 1.0 iff tq - tk >= 0, i.e. -x + y - 128*m >= 0
            nc.gpsimd.affine_select(
                out=masks[:, m, :], in_=masks[:, m, :],
                compare_op=mybir.AluOpType.is_ge, fill=0.0,
                base=-128 * m, pattern=[[1, 512]], channel_multiplier=-1)

        for l in range(L):
            _emit_layer(nc, pools, d, x_T, eps_sb, ones_bf, masks, l,
                        TS_FULL if l < L - 1 else TS_LAST, onesf)

        # final LN on last token only
        lnf = pools["sm"].tile([128, 12], F32, tag="lnf", name="lnf")
        nc.sync.dma_start(out=lnf[:, 0:6], in_=d["lnfw"])
        nc.sync.dma_start(out=lnf[:, 6:12], in_=d["lnfb"])
        hf = pools["act"].tile([128, KT, T], BF, tag="h", bufs=1, name="hf")
        _emit_ln(nc, pools, x_T, lnf, 0, lnf, 6, hf, TS_LAST, eps_sb, ones_bf, onesf)

        # lm head: [1,768] @ [768, 50688]
        for c in range(NVC):
            wt = pools["w"].tile([128, KT, 512], BF, tag="w", name="wt")
            nc.sync.dma_start(out=wt, in_=d["wteT"][c])
            ps = pools["sp"].tile([1, 512], F32, tag="sp", name="ps_lm")
            for kt in range(KT):
                nc.tensor.matmul(ps[0:1, :], hf[:, kt, 1023:1024], wt[:, kt, :],
                                 start=(kt == 0), stop=(kt == KT - 1))
            ob = pools["stat"].tile([1, 512], F32, tag="ob", bufs=2, name="ob")
            nc.vector.tensor_copy(out=ob, in_=ps[0:1, :])
            nc.sync.dma_start(out=logits_d[0:1, c * 512:(c + 1) * 512], in_=ob)

    nc.compile()
    return nc


_prog = None


def _get_prog():
    global _prog
    if _prog is None:
        _prog = _build_program()
    return _prog


def _make_in_maps(inputs):
    dev = _prep_weights(inputs)
    idx = np.asarray(inputs["idx"])
    wte = np.asarray(inputs["wte"], np.float32)
    wpe = np.asarray(inputs["wpe"], np.float32)
    x0 = wte[idx] + wpe[None, :, :]                     # [8,1024,768] fp32
    x0 = np.ascontiguousarray(
        x0.transpose(0, 2, 1).reshape(B, KT, 128, T).transpose(0, 2, 1, 3)
    )                                                   # [8,128,6,1024]
    in_maps = []
    for b in range(B):
        m = {"x0": x0[b]}
        m.update(dev)
        in_maps.append(m)
    return in_maps


def _assemble(results, inputs):
    lm_b = np.asarray(inputs["lm_b"], np.float32)
    out = np.stack([results[b]["logits"][0, :V] for b in range(B)])
    out = out + lm_b[None, :]
    return out.reshape(B, 1, V).astype(np.float32)


def kernel(**inputs):
    in_maps = _make_in_maps(inputs)
    nc = _get_prog()
    res = run_bass_kernel_spmd(nc, in_maps, list(range(B)))
    return _assemble(res.results, inputs)
